# revision 1
# baseline (speedup 1.0000x reference)
"""GCN (2-layer GCNConv + mean-pool + linear) on 8 Trainium2 NeuronCores.

Strategy (feature-major, scan-based segment reduction):
  - dst-shard nodes across 8 cores (12544 each, padded to 100352); edges live on
    their dst core, grouped into 8 gpsimd groups by src chunk, dst-sorted within.
  - y = dinv * x built per core, exchanged via AllGather into an SBUF-resident
    feature-major table [128, 12544] (group k rows 16k+f = y^T[f, chunk k]).
  - per chunk: gpsimd ap_gather -> per-edge src features; * edge weight; DVE
    prefix scan along edges; extract per-node run boundaries (indirect_copy);
    diff -> per-group partial sums; merge groups with a PE selector matmul.
  - layer 2 propagates one scalar per node: mean_pool(A h W2) @ Wlin
    == mean_pool(A (h (W2 @ Wlin))), so only z = h1 @ (W2@Wlin) flows.
  - pooling: prefix scan of q^T + extraction at graph ends; AllReduce [256].
All floating-point math runs on device; the host only sorts/permutes indices,
pads with zeros/ones, and builds constant 0/1 selector matrices.
"""
import numpy as np

N = 100000
NC = 8
NPC = 12544
NPAD = NC * NPC
NBLK = 98
S = 14
M = NPC // S          # 896
B = 256
F = 10
ECOLS = -(-(M + 1) // 16)  # 57
NPOOL = 64


def _ceil16(v):
    return ((v + 15) // 16) * 16


def _wrap16(vals, ncols, pad=0):
    v = np.asarray(vals)
    buf = np.full(ncols * 16, pad, dtype=v.dtype if len(v) else np.int16)
    buf[: len(v)] = v
    return buf.reshape(ncols, 16).T.copy()


def prep(x, edge_index, edge_weight, batch):
    """Pure index/permutation prep. Returns (per-core input dicts, meta)."""
    src = np.asarray(edge_index[0], dtype=np.int64)
    dst = np.asarray(edge_index[1], dtype=np.int64)
    w = np.asarray(edge_weight, dtype=np.float32)
    batch = np.asarray(batch, dtype=np.int64)
    x = np.asarray(x, dtype=np.float32)

    loops = np.arange(N, dtype=np.int64)
    src_all = np.concatenate([src, loops])
    dst_all = np.concatenate([dst, loops])
    w_all = np.concatenate([w, np.ones(N, dtype=np.float32)])

    g_all = src_all // NPC
    core_all = dst_all // NPC
    chunk_all = (dst_all % NPC) // M
    cell = ((core_all * NC + g_all) * S + chunk_all).astype(np.int64)
    C_ch = _ceil16(int(np.bincount(cell, minlength=NC * NC * S).max()) + 2)
    DP = int(np.bincount(dst_all, minlength=N).max())

    cnt = np.maximum(np.bincount(batch, minlength=B), 1).astype(np.int32)

    # constant 0/1 selector matrices (structural, data-independent)
    sel = np.zeros((128, 16), dtype=np.float32)
    sel[np.arange(128), np.arange(128) % 16] = 1.0
    sel2 = np.zeros((128, 1), dtype=np.float32)
    sel2[::16, 0] = 1.0

    cores = []
    for c in range(NC):
        lo = c * NPC
        hi = min((c + 1) * NPC, N)
        nreal = hi - lo

        xpk = np.zeros((128, NBLK * 11), dtype=np.float32)
        xl = np.zeros((NPC, F), dtype=np.float32)
        xl[:nreal] = x[lo:hi]
        for b_ in range(NBLK):
            xpk[:, b_ * 11 + 1 : b_ * 11 + 1 + F] = xl[b_ * 128 : (b_ + 1) * 128]

        mask = (dst_all >= lo) & (dst_all < hi)
        es, ed, ew = src_all[mask], (dst_all[mask] - lo).astype(np.int64), w_all[mask]
        eg = es // NPC
        esl = (es - eg * NPC).astype(np.int16)

        w2pad = np.zeros((128, NBLK * DP), dtype=np.float32)
        order_d = np.argsort(ed, kind="stable")
        d_sorted, w_sorted = ed[order_d], ew[order_d]
        node_starts = np.searchsorted(d_sorted, np.arange(NPC + 1))
        p_of = np.arange(NPC) % 128
        b_of = np.arange(NPC) // 128
        lens = np.diff(node_starts)
        for l in np.nonzero(lens)[0]:
            a = node_starts[l]
            w2pad[p_of[l], b_of[l] * DP : b_of[l] * DP + lens[l]] = w_sorted[a : a + lens[l]]
        for l in range(nreal, NPC):  # pad nodes: deg = 1 so dinv stays finite
            w2pad[p_of[l], b_of[l] * DP] = 1.0

        gidx = np.zeros((128, S * (C_ch // 16)), dtype=np.int16)
        wrep = np.zeros((128, S * C_ch), dtype=np.float32)
        eidx = np.zeros((128, S * ECOLS), dtype=np.uint16)
        order = np.lexsort((ed, eg))
        gs, ds, ws, sls = eg[order], ed[order], ew[order], esl[order]
        grp_starts = np.searchsorted(gs, np.arange(NC + 1))
        for k in range(NC):
            ga, gb = grp_starts[k], grp_starts[k + 1]
            dk, wk, slk = ds[ga:gb], ws[ga:gb], sls[ga:gb]
            chunk_starts = np.searchsorted(dk, np.arange(0, NPC + M, M))
            for j in range(S):
                ca, cb = chunk_starts[j], chunk_starts[j + 1]
                n_e = cb - ca
                idx_slots = np.zeros(C_ch, dtype=np.int16)
                idx_slots[1 : 1 + n_e] = slk[ca:cb]
                w_slots = np.zeros(C_ch, dtype=np.float32)
                w_slots[1 : 1 + n_e] = wk[ca:cb]
                gidx[16 * k : 16 * (k + 1), j * (C_ch // 16) : (j + 1) * (C_ch // 16)] = (
                    idx_slots.reshape(C_ch // 16, 16).T
                )
                wrep[16 * k : 16 * (k + 1), j * C_ch : (j + 1) * C_ch] = w_slots[None, :]
                ends = np.zeros(M + 1, dtype=np.uint16)
                ends[1:] = np.searchsorted(dk[ca:cb], np.arange(j * M, (j + 1) * M), side="right").astype(np.uint16)
                epad = np.zeros(ECOLS * 16, dtype=np.uint16)
                epad[: M + 1] = ends
                eidx[16 * k : 16 * (k + 1), j * ECOLS : (j + 1) * ECOLS] = epad.reshape(ECOLS, 16).T

        gmin, gmax = int(batch[lo]), int(batch[hi - 1])
        glist = np.arange(gmin, gmax + 1)
        gends = np.searchsorted(batch, glist, side="right") - lo
        gends = np.minimum(gends, NPC).astype(np.int16)
        n_gc = len(glist)
        pool_end_vals = np.full(NPOOL, gends[-1] - 1, dtype=np.int16)
        pool_end_vals[:n_gc] = gends - 1
        pool_end = _wrap16(pool_end_vals, 4)
        place = np.full(B, NPOOL, dtype=np.int16)
        place[gmin : gmax + 1] = np.arange(n_gc, dtype=np.int16)
        pool_place = _wrap16(place, 16)

        cores.append(
            dict(
                xpk=xpk, w2pad=w2pad, gidx=gidx, wrep=wrep, eidx=eidx,
                pool_end=pool_end.astype(np.int16), pool_place=pool_place.astype(np.int16),
                cnt=cnt.reshape(1, B), sel=sel, sel2=sel2,
            )
        )
    return cores, dict(C_ch=C_ch, DP=DP)


# ------------------------------------------------------------------ device
def build_program(C_ch, DP):
    import concourse.bass as bass
    import concourse.bacc as bacc
    import concourse.mybir as mybir
    import concourse.tile as tile
    from concourse.masks import make_identity

    f32 = mybir.dt.float32
    i16 = mybir.dt.int16
    u16 = mybir.dt.uint16
    i32 = mybir.dt.int32
    AX = mybir.AxisListType.X
    OP = mybir.AluOpType
    AF = mybir.ActivationFunctionType

    nc = bacc.Bacc("TRN2", target_bir_lowering=False, debug=False, num_devices=NC)

    def din(name, shape, dt=f32):
        return nc.dram_tensor(name, shape, dt, kind="ExternalInput")

    xpk_d = din("xpk", [128, NBLK * 11])
    w2_d = din("w2pad", [128, NBLK * DP])
    gidx_d = din("gidx", [128, S * (C_ch // 16)], i16)
    wrep_d = din("wrep", [128, S * C_ch])
    eidx_d = din("eidx", [128, S * ECOLS], u16)
    pend_d = din("pool_end", [16, 4], i16)
    pplace_d = din("pool_place", [16, 16], i16)
    cnt_d = din("cnt", [1, B], i32)
    sel_d = din("sel", [128, 16])
    sel2_d = din("sel2", [128, 1])
    W1_d = din("W1", [F, 128])
    b1_d = din("b1", [128, 1])
    W2_d = din("W2", [128, 128])
    wlr_d = din("wlin_row", [1, 128])
    wlc_d = din("wlin_col", [128, 1])
    blin_d = din("blin", [1, 1])
    b2_d = din("b2row", [1, 128])
    out_d = nc.dram_tensor("out", [1, B], f32, kind="ExternalOutput")

    rg = [list(range(NC))]

    with tile.TileContext(nc) as tc:
        from contextlib import ExitStack

        with ExitStack() as ctx:
            sb = ctx.enter_context(tc.tile_pool(name="sb", bufs=1))
            big = ctx.enter_context(tc.tile_pool(name="big", bufs=1))
            dram = ctx.enter_context(tc.tile_pool(name="dram", bufs=1, space="DRAM"))
            gpool = ctx.enter_context(tc.tile_pool(name="gp", bufs=2))
            wpool = ctx.enter_context(tc.tile_pool(name="wp", bufs=1))
            mpool = ctx.enter_context(tc.tile_pool(name="mp", bufs=1))
            epool = ctx.enter_context(tc.tile_pool(name="ep", bufs=2))
            ppool = ctx.enter_context(tc.tile_pool(name="pp", bufs=2))
            tpool = ctx.enter_context(tc.tile_pool(name="tp", bufs=1))
            hpool = ctx.enter_context(tc.tile_pool(name="hp", bufs=2))
            dpool = ctx.enter_context(tc.tile_pool(name="dp", bufs=1))

            # --- constants
            selt = sb.tile([128, 16], f32)
            nc.sync.dma_start(out=selt[:], in_=sel_d[:, :])
            sel2t = sb.tile([128, 1], f32)
            nc.sync.dma_start(out=sel2t[:], in_=sel2_d[:, :])
            W1t = sb.tile([F, 128], f32)
            nc.sync.dma_start(out=W1t[:], in_=W1_d[:, :])
            b1t = sb.tile([128, 1], f32)
            nc.sync.dma_start(out=b1t[:], in_=b1_d[:, :])
            wlrt = sb.tile([1, 128], f32)
            nc.sync.dma_start(out=wlrt[:], in_=wlr_d[:, :])
            wlct = sb.tile([128, 1], f32)
            nc.sync.dma_start(out=wlct[:], in_=wlc_d[:, :])
            blint = sb.tile([1, 1], f32)
            nc.sync.dma_start(out=blint[:], in_=blin_d[:, :])
            b2t = sb.tile([1, 128], f32)
            nc.sync.dma_start(out=b2t[:], in_=b2_d[:, :])
            pendt = sb.tile([16, 4], i16)
            nc.sync.dma_start(out=pendt[:], in_=pend_d[:, :])
            pplacet = sb.tile([16, 16], i16)
            nc.sync.dma_start(out=pplacet[:], in_=pplace_d[:, :])
            cntt = sb.tile([1, B], i32)
            nc.sync.dma_start(out=cntt[:], in_=cnt_d[:, :])
            zerot = sb.tile([128, 1], f32)
            nc.vector.memset(zerot[:], 0.0)
            ones10 = sb.tile([1, 16], f32)
            nc.vector.memset(ones10[:], 1.0)

            # --- phase A: deg, dinv, y (in place on xpk), transposes
            wz = sb.tile([128, 1], f32)
            with tc.tile_pool(name="pha", bufs=1) as pha, \
                 tc.tile_pool(name="pst", bufs=2, space="PSUM") as pst:
                # wz = W2 @ Wlin via PE: transpose W2, then (W2^T).T @ wlin_col
                ident = pha.tile([128, 128], f32)
                make_identity(nc, ident[:])
                W2t = pha.tile([128, 128], f32)
                nc.sync.dma_start(out=W2t[:], in_=W2_d[:, :])
                w2tp = pst.tile([128, 512], f32, tag="pt")
                nc.tensor.transpose(out=w2tp[:, :128], in_=W2t[:], identity=ident[:])
                w2ts = pha.tile([128, 128], f32)
                nc.scalar.copy(out=w2ts[:], in_=w2tp[:, :128])
                wzp = pst.tile([128, 512], f32, tag="pt")
                nc.tensor.matmul(out=wzp[:, :1], lhsT=w2ts[:], rhs=wlct[:], start=True, stop=True)
                nc.scalar.copy(out=wz[:], in_=wzp[:, :1])
                xpkt = pha.tile([128, NBLK * 11], f32)
                nc.sync.dma_start(out=xpkt[:], in_=xpk_d[:, :])
                deg = pha.tile([128, NBLK], f32)
                HB = NBLK // 7
                for hh in range(7):
                    w2t_ = pha.tile([128, HB * DP], f32, tag="w2t", name=f"w2t{hh}")
                    nc.sync.dma_start(out=w2t_[:], in_=w2_d[:, hh * HB * DP : (hh + 1) * HB * DP])
                    nc.vector.tensor_reduce(
                        out=deg[:, hh * HB : (hh + 1) * HB],
                        in_=w2t_[:].rearrange("p (b d) -> p b d", d=DP), axis=AX, op=OP.add
                    )
                dinv = pha.tile([128, NBLK], f32)
                nc.scalar.activation(out=deg[:], in_=deg[:], func=AF.Sqrt)
                nc.vector.reciprocal(out=dinv[:], in_=deg[:])
                xv = xpkt[:].rearrange("p (b f) -> p b f", f=11)
                dv = dinv[:].rearrange("p (b o) -> p b o", o=1)
                nc.vector.tensor_tensor(
                    out=xv[:, :, 1 : F + 1], in0=xv[:, :, 1 : F + 1],
                    in1=dv.to_broadcast([128, NBLK, F]), op=OP.mult
                )
                nc.vector.tensor_copy(out=xv[:, :, 0:1], in_=dv)

                glob = big.tile([128, NPC], f32)
                nc.scalar.activation(
                    out=glob[0:16, :], in_=zerot[0:16, :].to_broadcast([16, NPC]), func=AF.Copy
                )
                for b4 in range(25):
                    nb = min(4, NBLK - b4 * 4)
                    ptile = pst.tile([128, 512], f32, tag="pt", name=f"ptile{b4}")
                    for bb in range(nb):
                        b_ = b4 * 4 + bb
                        nc.tensor.transpose(
                            out=ptile[0:11, bb * 128 : (bb + 1) * 128],
                            in_=xv[:, b_, :],
                            identity=ident[:],
                        )
                    nc.scalar.copy(
                        out=glob[0:11, b4 * 512 : b4 * 512 + nb * 128],
                        in_=ptile[0:11, : nb * 128],
                    )

            # --- AllGather y
            yag_in = dram.tile([F, NPC], f32)
            yag_out = dram.tile([NC, F * NPC], f32)
            nc.sync.dma_start(out=yag_in[:], in_=glob[1 : F + 1, :])
            nc.gpsimd.collective_compute(
                "AllGather", mybir.AluOpType.bypass, replica_groups=rg,
                ins=[yag_in[:]], outs=[yag_out[:]],
            )
            table = big.tile([128, NPC], f32)
            nc.scalar.activation(
                out=table[:], in_=zerot[:].to_broadcast([128, NPC]), func=AF.Copy
            )
            yag_v = yag_out[:].rearrange("k (f n) -> k f n", f=F)
            for k in range(NC):
                nc.sync.dma_start(out=table[16 * k : 16 * k + F, :], in_=yag_v[k])

            psm = ctx.enter_context(tc.tile_pool(name="psm", bufs=2, space="PSUM"))
            psb = ctx.enter_context(tc.tile_pool(name="psb", bufs=2, space="PSUM"))

            # --- shared chunk pipeline (layers 1 and 2); gathers batched per
            # chunk-PAIR (halves Pool-engine gather dispatches)
            GC = C_ch // 16

            WSTART = {0: 4, 4: 4, 8: 4, 12: 2}  # chunk windows for batched gathers

            def edge_win(j0, nchunk, lyr):
                gix = gpool.tile([128, 4 * GC], i16, tag="gix", name=f"gix{lyr}_{j0}")
                nc.sync.dma_start(out=gix[:, : nchunk * GC], in_=gidx_d[:, j0 * GC : (j0 + nchunk) * GC])
                msgs = mpool.tile([128, 4 * C_ch], f32, tag="msgs", name=f"msgs{lyr}_{j0}")
                nc.gpsimd.ap_gather(
                    out_ap=msgs[:, : nchunk * C_ch], in_ap=table[:], idxs_ap=gix[:, : nchunk * GC],
                    channels=128, num_elems=NPC, d=1, num_idxs=nchunk * C_ch,
                )
                for hw_ in range(nchunk // 2):
                    wre = wpool.tile([128, 2 * C_ch], f32, tag="wre", name=f"wre{lyr}_{j0}_{hw_}")
                    nc.sync.dma_start(
                        out=wre[:],
                        in_=wrep_d[:, (j0 + 2 * hw_) * C_ch : (j0 + 2 * hw_ + 2) * C_ch],
                    )
                    nc.vector.tensor_tensor(
                        out=msgs[:, 2 * hw_ * C_ch : (2 * hw_ + 2) * C_ch],
                        in0=msgs[:, 2 * hw_ * C_ch : (2 * hw_ + 2) * C_ch],
                        in1=wre[:], op=OP.mult,
                    )
                return msgs

            def chunk_tail(msgs, js, j):
                sc = msgs[:, js * C_ch : (js + 1) * C_ch]
                nc.vector.tensor_tensor_scan(
                    out=sc, data0=sc,
                    data1=zerot[:].to_broadcast([128, C_ch]),
                    initial=0.0, op0=OP.add, op1=OP.add,
                )
                eix = epool.tile([128, ECOLS], u16, tag="eix")
                nc.sync.dma_start(out=eix[:], in_=eidx_d[:, j * ECOLS : (j + 1) * ECOLS])
                E = epool.tile([128, ECOLS * 16], f32, tag="E")
                nc.gpsimd.indirect_copy(
                    out=E[:, : M + 1], data=sc, idxs=eix[:], i_know_ap_gather_is_preferred=True
                )
                Pd = ppool.tile([128, M], f32, tag="Pd")
                nc.vector.tensor_tensor(out=Pd[:], in0=E[:, 1 : M + 1], in1=E[:, 0:M], op=OP.subtract)
                return Pd

            # --- layer 1
            jw1 = 0
            for j in range(S):
                if j in WSTART:
                    msgs_l1 = edge_win(j, WSTART[j], 1)
                    jw1 = j
                Pd = chunk_tail(msgs_l1, j - jw1, j)
                pT = tpool.tile([F, M], f32, tag="pT")
                for h in range(2):
                    pm_ = psm.tile([F, 448], f32, tag="pm")
                    nc.tensor.matmul(
                        out=pm_[:], lhsT=selt[:, 0:F], rhs=Pd[:, h * 448 : (h + 1) * 448],
                        start=True, stop=True,
                    )
                    dvp = psb.tile([F, 448], f32, tag="dvp", name=f"dvp{j}_{h}")
                    nc.tensor.matmul(
                        out=dvp[:], lhsT=ones10[:, 0:F],
                        rhs=glob[0:1, j * M + h * 448 : j * M + (h + 1) * 448],
                        start=True, stop=True,
                    )
                    dvs = dpool.tile([F, 448], f32, tag="dvs", name=f"dvs{j}_{h}")
                    nc.scalar.copy(out=dvs[:], in_=dvp[:])
                    nc.vector.tensor_tensor(
                        out=pT[:, h * 448 : (h + 1) * 448], in0=pm_[:], in1=dvs[:], op=OP.mult,
                    )
                zps = [psb.tile([1, 512], f32, tag="zp", name=f"zp{j}_{h_}") for h_ in range(2)]
                for bb in range(7):
                    st = psm.tile([128, 128], f32, tag="st")
                    nc.tensor.matmul(
                        out=st[:], lhsT=W1t[:], rhs=pT[:, bb * 128 : (bb + 1) * 128],
                        start=True, stop=True,
                    )
                    ht = hpool.tile([128, 128], f32, tag="ht")
                    nc.scalar.activation(out=ht[:], in_=st[:], func=AF.Relu, bias=b1t[:])
                    zp = zps[bb // 4]
                    nc.tensor.matmul(
                        out=zp[:, (bb % 4) * 128 : (bb % 4) * 128 + 128],
                        lhsT=wz[:], rhs=ht[:], start=True, stop=True,
                    )
                for h, n_ in ((0, 512), (1, 384)):
                    nc.vector.tensor_tensor(
                        out=glob[32:33, j * M + h * 512 : j * M + h * 512 + n_],
                        in0=zps[h][:, :n_],
                        in1=glob[0:1, j * M + h * 512 : j * M + h * 512 + n_],
                        op=OP.mult,
                    )

            # --- AllGather zy; z-table rows 16k
            zag_in = dram.tile([1, NPC], f32)
            zag_out = dram.tile([NC, NPC], f32)
            nc.sync.dma_start(out=zag_in[:], in_=glob[32:33, :])
            nc.gpsimd.collective_compute(
                "AllGather", mybir.AluOpType.bypass, replica_groups=rg,
                ins=[zag_in[:]], outs=[zag_out[:]],
            )
            for k in range(NC):
                nc.sync.dma_start(out=table[16 * k : 16 * k + 1, :], in_=zag_out[k : k + 1, :])

            # --- layer 2
            jw2 = 0
            for j in range(S):
                if j in WSTART:
                    msgs_l2 = edge_win(j, WSTART[j], 2)
                    jw2 = j
                Pd = chunk_tail(msgs_l2, j - jw2, j)
                for h in range(2):
                    n_ = 448 if h == 0 else 448
                    qm = psb.tile([1, 512], f32, tag="zp", name=f"qm{j}_{h}")
                    nc.tensor.matmul(
                        out=qm[:, :448], lhsT=sel2t[:], rhs=Pd[:, h * 448 : (h + 1) * 448],
                        start=True, stop=True,
                    )
                    nc.vector.tensor_tensor(
                        out=glob[64:65, j * M + h * 448 : j * M + (h + 1) * 448],
                        in0=qm[:, :448],
                        in1=glob[0:1, j * M + h * 448 : j * M + (h + 1) * 448],
                        op=OP.mult,
                    )

            # --- pooling
            nc.vector.tensor_tensor_scan(
                out=glob[0:1, :], data0=glob[64:65, :],
                data1=zerot[64:65, :].to_broadcast([1, NPC]),
                initial=0.0, op0=OP.add, op1=OP.add,
            )
            Ep = sb.tile([16, NPOOL], f32)
            nc.gpsimd.ap_gather(
                out_ap=Ep[:], in_ap=glob[0:16, :], idxs_ap=pendt[:],
                channels=16, num_elems=NPC, d=1, num_idxs=NPOOL,
            )
            Pp = sb.tile([16, NPOOL + 1], f32)
            nc.vector.memset(Pp[:], 0.0)
            nc.vector.tensor_copy(out=Pp[0:1, 0:1], in_=Ep[0:1, 0:1])
            nc.vector.tensor_tensor(
                out=Pp[0:1, 1:NPOOL], in0=Ep[0:1, 1:NPOOL], in1=Ep[0:1, 0 : NPOOL - 1],
                op=OP.subtract,
            )
            placed = sb.tile([16, B], f32)
            nc.gpsimd.ap_gather(
                out_ap=placed[:], in_ap=Pp[:], idxs_ap=pplacet[:],
                channels=16, num_elems=NPOOL + 1, d=1, num_idxs=B,
            )

            par_in = dram.tile([1, B], f32)
            par_out = dram.tile([1, B], f32)
            nc.sync.dma_start(out=par_in[:], in_=placed[0:1, :])
            nc.gpsimd.collective_compute(
                "AllReduce", mybir.AluOpType.add, replica_groups=rg,
                ins=[par_in[:]], outs=[par_out[:]],
            )
            art = sb.tile([1, B], f32)
            nc.sync.dma_start(out=art[:], in_=par_out[:])

            cntf = sb.tile([1, B], f32)
            nc.vector.tensor_copy(out=cntf[:], in_=cntt[:])
            rec = sb.tile([1, B], f32)
            nc.vector.reciprocal(out=rec[:], in_=cntf[:])
            res = sb.tile([1, B], f32)
            nc.vector.tensor_tensor(out=res[:], in0=art[:], in1=rec[:], op=OP.mult)
            cb = sb.tile([1, 128], f32)
            nc.vector.tensor_tensor(out=cb[:], in0=b2t[:], in1=wlrt[:], op=OP.mult)
            cs = sb.tile([1, 1], f32)
            nc.vector.tensor_reduce(out=cs[:], in_=cb[:], axis=AX, op=OP.add)
            nc.vector.tensor_tensor(out=cs[:], in0=cs[:], in1=blint[:], op=OP.add)
            nc.vector.tensor_tensor(
                out=res[:], in0=res[:], in1=cs[:].to_broadcast([1, B]), op=OP.add
            )
            nc.sync.dma_start(out=out_d[:, :], in_=res[:])

    nc.compile()
    return nc


_CACHE = {}


def kernel(**inputs):
    from concourse.bass_utils import run_bass_kernel_spmd

    cores, meta = prep(
        inputs["x"], inputs["edge_index"], inputs["edge_weight"], inputs["batch"]
    )
    key = (meta["C_ch"], meta["DP"])
    if key not in _CACHE:
        _CACHE[key] = build_program(*key)
    nc = _CACHE[key]

    W1 = np.asarray(inputs["W1"], dtype=np.float32)
    b1 = np.asarray(inputs["b1"], dtype=np.float32).reshape(128, 1)
    W2 = np.asarray(inputs["W2"], dtype=np.float32)
    wlr = np.asarray(inputs["Wlin"], dtype=np.float32).reshape(1, 128)
    wlc = np.asarray(inputs["Wlin"], dtype=np.float32).reshape(128, 1)
    blin = np.asarray(inputs["blin"], dtype=np.float32).reshape(1, 1)
    b2r = np.asarray(inputs["b2"], dtype=np.float32).reshape(1, 128)

    in_maps = []
    for c in range(NC):
        cr = cores[c]
        in_maps.append(
            dict(
                xpk=cr["xpk"], w2pad=cr["w2pad"], gidx=cr["gidx"], wrep=cr["wrep"],
                eidx=cr["eidx"], pool_end=cr["pool_end"], pool_place=cr["pool_place"],
                cnt=cr["cnt"], sel=cr["sel"], sel2=cr["sel2"],
                W1=W1, b1=b1, W2=W2, wlin_row=wlr, wlin_col=wlc, blin=blin, b2row=b2r,
            )
        )
    res = run_bass_kernel_spmd(nc, in_maps, list(range(NC)))
    out = np.asarray(res.results[0]["out"], dtype=np.float32).reshape(B, 1)
    return out



# revision 12
# speedup vs baseline: 1.4648x; 1.4648x over previous
"""GCN (2-layer GCNConv + mean-pool + linear) on 8 Trainium2 NeuronCores.

Strategy (v2: fp16 edge pipeline, static parity subcells, pair-gathers):
  - dst-shard nodes across 8 cores (12544 each); self-loops REMOVED from edge
    lists (handled as a PSUM-accumulated matmul term against the feature
    table, selected by a per-core 0/1 matrix).
  - edges bucketed into static cells (src-chunk group k, dst chunk j, src
    parity) of C1 slots, dst-sorted within a cell; chunks processed in 3
    windows of 5/5/4; the window stream is [even subcells | odd subcells].
  - ap_gather moves 4-byte units, so the fp16 feature table [128, 12544]
    (group k rows 16k+f hold y^T[f] = dinv*x; row 16k+10 holds layer-2's z')
    is gathered through its f32 bitcast with idx = src//2; strided fp16
    multiplies by the edge weights select the parity half and apply w ->
    in-place prefix scan over the window -> per-(chunk,parity) indirect_copy
    (<=1024 idxs per call: walrus ISA limit) extracts per-node boundary
    prefixes -> per-chunk merge via +/- selector matmuls accumulated in PSUM
    together with the self-loop term -> * dinv_dst -> W1 + relu + z (layer 1).
  - layer 2 propagates one scalar per node (z = h1 @ (W2 @ Wlin)); mean pool
    via a DRAM roundtrip into a [128, 98] block layout, per-partition scan +
    triangular-matmul offsets, tiny ap_gather of graph ends, masked
    partition-collapse matmul, and AllGather + ones-matmul instead of an
    AllReduce.
All floating-point math runs on device; the host only sorts/permutes indices,
pads with zeros/ones, and builds constant 0/+-1 selector matrices.
"""
import numpy as np

N = 100000
NC = 8
NPC = 12544
B = 256
F = 10
S = 14
M = NPC // S          # 896
ROWL = NPC // 128     # 98
NW = 3
CST = [0, 5, 10]      # window chunk starts
CPWS = [5, 5, 4]      # chunks per window
NPOOL = 64
EBC = 928             # boundary slots per (chunk, parity): 4B-aligned idx slices
EW = EBC // 16        # 58


def _ceil16(v):
    return ((v + 15) // 16) * 16


def _wrap16(vals):
    v = np.asarray(vals)
    assert len(v) % 16 == 0
    return v.reshape(len(v) // 16, 16).T.copy()


def prep(x, edge_index, edge_weight, batch):
    """Pure index/permutation prep. Returns (per-core input dicts, meta)."""
    src = np.asarray(edge_index[0], dtype=np.int64)
    dst = np.asarray(edge_index[1], dtype=np.int64)
    w = np.asarray(edge_weight, dtype=np.float32)
    batch = np.asarray(batch, dtype=np.int64)
    x = np.asarray(x, dtype=np.float32)

    DP = 1 + int(np.bincount(dst, minlength=N).max())

    core_e = dst // NPC
    g_all = src // NPC
    dstloc = dst - core_e * NPC
    chunk_all = dstloc // M
    par_all = src % 2
    cell = ((core_e * NC + g_all) * S + chunk_all) * 2 + par_all
    C1 = _ceil16(int(np.bincount(cell, minlength=NC * NC * S * 2).max()) + 1)
    TSL = S * 2 * C1              # total stream slots per group
    WOFF = [0, 10 * C1, 20 * C1]  # window stream offsets
    GL = TSL // 16

    cnt = np.maximum(np.bincount(batch, minlength=B), 1).astype(np.int32)

    sel16 = np.zeros((128, 16), dtype=np.float16)
    sel16[np.arange(128), np.arange(128) % 16] = 1.0
    negsel16 = (-sel16).astype(np.float16)
    sel2 = np.zeros((128, 1), dtype=np.float16)
    sel2[10::16] = 1.0
    negsel2 = (-sel2).astype(np.float16)
    tri = np.zeros((128, 128), dtype=np.float32)
    tri[np.triu_indices(128, 1)] = 1.0
    ones8 = np.ones((8, 1), dtype=np.float32)
    ones128 = np.ones((128, 1), dtype=np.float32)

    cores = []
    for c in range(NC):
        lo = c * NPC
        hi = min((c + 1) * NPC, N)
        nreal = hi - lo

        xT = np.zeros((16, NPC), dtype=np.float32)
        xT[:F, :nreal] = x[lo:hi].T

        # weighted-degree pad: node n -> (p=n//98, c2=n%98), slot 0 = self w=1
        w2 = np.zeros((128, ROWL, DP), dtype=np.float32)
        w2[:, :, 0] = 1.0
        emask = (dst >= lo) & (dst < hi)
        es, ed, ew = src[emask], (dst[emask] - lo), w[emask]
        od = np.argsort(ed, kind="stable")
        ed_s, ew_s = ed[od], ew[od]
        starts = np.searchsorted(ed_s, np.arange(NPC))
        rank = np.arange(len(ed_s)) - starts[ed_s]
        w2[ed_s // ROWL, ed_s % ROWL, 1 + rank] = ew_s
        DPA = (DP + 1) // 2
        w2a = w2[:, :, :DPA].reshape(128, ROWL * DPA).copy()
        w2b = w2[:, :, DPA:].reshape(128, ROWL * (DP - DPA)).copy()

        eg = es // NPC
        esl = es - eg * NPC
        echunk = ed // M
        epar = esl % 2
        gidx = np.zeros((128, GL), dtype=np.int16)
        wrep = np.zeros((128, TSL), dtype=np.float32)
        eidx = np.zeros((128, S * 2 * EW), dtype=np.uint16)
        for k in range(NC):
            idx_slots = np.zeros(TSL, dtype=np.int16)
            w_slots = np.zeros(TSL, dtype=np.float32)
            for wdw in range(NW):
                cpw = CPWS[wdw]
                for jl in range(cpw):
                    j = CST[wdw] + jl
                    for par in range(2):
                        m = (eg == k) & (echunk == j) & (epar == par)
                        dk, wk, sk = ed[m], ew[m], esl[m] // 2
                        o = np.argsort(dk, kind="stable")
                        dk, wk, sk = dk[o], wk[o], sk[o]
                        n_e = len(dk)
                        assert n_e + 1 <= C1
                        base = WOFF[wdw] + (par * cpw + jl) * C1
                        idx_slots[base + 1 : base + 1 + n_e] = sk
                        w_slots[base + 1 : base + 1 + n_e] = wk
                        bounds = np.zeros(EBC, dtype=np.uint16)
                        bounds[: M + 1] = np.searchsorted(
                            dk, np.arange(j * M, j * M + M + 1)
                        ).astype(np.uint16)
                        eidx[16 * k : 16 * (k + 1),
                             (j * 2 + par) * EW : (j * 2 + par + 1) * EW] = _wrap16(bounds)
            gidx[16 * k : 16 * (k + 1), :] = _wrap16(idx_slots)
            wrep[16 * k : 16 * (k + 1), :] = w_slots[None, :]

        selfsel = np.zeros((128, 16), dtype=np.float16)
        selfsel[16 * c + np.arange(16), np.arange(16)] = 1.0
        selfsel2 = np.zeros((128, 1), dtype=np.float16)
        selfsel2[16 * c + 10] = 1.0

        gmin, gmax = int(batch[lo]), int(batch[hi - 1])
        glist = np.arange(gmin, gmax + 1)
        n_gc = len(glist)
        assert n_gc <= NPOOL
        gends = np.minimum(np.searchsorted(batch, glist, side="right") - lo, NPC)
        ends_node = np.maximum(gends - 1, 0)
        p_i = (ends_node // ROWL).astype(np.int64)
        c_i = (ends_node % ROWL).astype(np.int16)
        vals_by_group = np.zeros((NC, NPOOL), dtype=np.int16)
        vals_by_group[p_i // 16, np.arange(n_gc)] = c_i
        poolidx = np.zeros((128, NPOOL // 16), dtype=np.int16)
        for G in range(NC):
            poolidx[16 * G : 16 * (G + 1), :] = _wrap16(vals_by_group[G])
        maskp = np.zeros((128, NPOOL), dtype=np.float32)
        maskp[p_i, np.arange(n_gc)] = 1.0
        place_vals = np.full(B, NPOOL, dtype=np.int16)
        place_vals[gmin : gmax + 1] = np.arange(n_gc, dtype=np.int16)
        place = _wrap16(place_vals)

        cores.append(
            dict(
                xT=xT, w2a=w2a, w2b=w2b, gidx=gidx, wrep=wrep, eidx=eidx,
                sel16=sel16, negsel16=negsel16, selfsel=selfsel,
                sel2=sel2, negsel2=negsel2, selfsel2=selfsel2,
                tri=tri, ones8=ones8, ones128=ones128,
                poolidx=poolidx, maskp=maskp, place=place,
                cnt=cnt.reshape(1, B),
            )
        )
    return cores, dict(C1=C1, DP=DP)


# ------------------------------------------------------------------ device
def build_program(C1, DP):
    import concourse.bass as bass
    import concourse.bacc as bacc
    import concourse.mybir as mybir
    import concourse.tile as tile
    from concourse.masks import make_identity

    f32 = mybir.dt.float32
    f16 = mybir.dt.float16
    i16 = mybir.dt.int16
    u16 = mybir.dt.uint16
    i32 = mybir.dt.int32
    AX = mybir.AxisListType.X
    OP = mybir.AluOpType
    AF = mybir.ActivationFunctionType

    TSL = S * 2 * C1
    WOFF = [0, 10 * C1, 20 * C1]
    WLEN = [10 * C1, 10 * C1, 8 * C1]
    GL = TSL // 16
    DPA = (DP + 1) // 2
    DPB = DP - DPA

    nc = bacc.Bacc("TRN2", target_bir_lowering=False, debug=False, num_devices=NC)

    def din(name, shape, dt=f32):
        return nc.dram_tensor(name, shape, dt, kind="ExternalInput")

    xT_d = din("xT", [16, NPC])
    w2a_d = din("w2a", [128, ROWL * DPA])
    w2b_d = din("w2b", [128, ROWL * DPB])
    gidx_d = din("gidx", [128, GL], i16)
    wrep_d = din("wrep", [128, TSL])
    eidx_d = din("eidx", [128, S * 2 * EW], u16)
    sel16_d = din("sel16", [128, 16], f16)
    negsel16_d = din("negsel16", [128, 16], f16)
    selfsel_d = din("selfsel", [128, 16], f16)
    sel2_d = din("sel2", [128, 1], f16)
    negsel2_d = din("negsel2", [128, 1], f16)
    selfsel2_d = din("selfsel2", [128, 1], f16)
    tri_d = din("tri", [128, 128])
    ones8_d = din("ones8", [8, 1])
    ones128_d = din("ones128", [128, 1])
    poolidx_d = din("poolidx", [128, NPOOL // 16], i16)
    maskp_d = din("maskp", [128, NPOOL])
    place_d = din("place", [16, 16], i16)
    cnt_d = din("cnt", [1, B], i32)
    W1_d = din("W1", [F, 128])
    b1_d = din("b1", [128, 1])
    W2_d = din("W2", [128, 128])
    wlc_d = din("wlin_col", [128, 1])
    blin_d = din("blin", [1, 1])
    b2_d = din("b2row", [1, 128])
    wlr_d = din("wlin_row", [1, 128])
    out_d = nc.dram_tensor("out", [1, B], f32, kind="ExternalOutput")

    rg = [list(range(NC))]

    with tile.TileContext(nc) as tc:
        from contextlib import ExitStack

        with ExitStack() as ctx:
            sb = ctx.enter_context(tc.tile_pool(name="sb", bufs=1))
            big = ctx.enter_context(tc.tile_pool(name="big", bufs=1))
            dram = ctx.enter_context(tc.tile_pool(name="dram", bufs=1, space="DRAM"))

            # ---- resident tiles
            gbuf = big.tile([128, 10 * C1], f32)
            msgsA = big.tile([128, 10 * C1], f16)
            msgsB = big.tile([128, 10 * C1], f16)
            wrep16 = big.tile([128, TSL], f16)
            table = big.tile([128, NPC], f16)
            dinvw = big.tile([16, 5 * M], f16)
            gidx_sb = big.tile([128, GL], i16)
            eidx_sb = big.tile([128, S * 2 * EW], u16)

            # ---- small constants / scratch
            sel16t = sb.tile([128, 16], f16)
            negsel16t = sb.tile([128, 16], f16)
            selfselt = sb.tile([128, 16], f16)
            sel2t = sb.tile([128, 1], f16)
            negsel2t = sb.tile([128, 1], f16)
            selfsel2t = sb.tile([128, 1], f16)
            trit = sb.tile([128, 128], f32)
            ones8t = sb.tile([8, 1], f32)
            ones128t = sb.tile([128, 1], f32)
            poolidxt = sb.tile([128, NPOOL // 16], i16)
            maskpt = sb.tile([128, NPOOL], f32)
            placet = sb.tile([16, 16], i16)
            cntt = sb.tile([1, B], i32)
            W1f = sb.tile([F, 128], f32)
            W1t = sb.tile([F, 128], f16)
            b1t = sb.tile([128, 1], f32)
            wlct = sb.tile([128, 1], f32)
            blint = sb.tile([1, 1], f32)
            b2t = sb.tile([1, 128], f32)
            wlrt = sb.tile([1, 128], f32)
            wzt = sb.tile([128, 1], f16)
            zerot16 = sb.tile([128, 1], f16)
            dega = sb.tile([128, ROWL], f32)
            degb = sb.tile([128, ROWL], f32)
            dinvt = sb.tile([128, ROWL], f16)
            qblk = sb.tile([128, ROWL], f16)
            qP = sb.tile([128, ROWL], f32)
            offs = sb.tile([128, 1], f32)
            ext = sb.tile([128, NPOOL], f32)
            masked = sb.tile([128, NPOOL], f32)
            Eps = sb.tile([1, NPOOL], f32)
            Pp = sb.tile([16, NPOOL + 1], f32)
            placed = sb.tile([16, B], f32)
            arp = sb.tile([8, B], f32)
            res = sb.tile([1, B], f32)
            cntf = sb.tile([1, B], f32)
            rec = sb.tile([1, B], f32)
            cb = sb.tile([1, 128], f32)
            cs = sb.tile([1, 1], f32)

            for t, d in (
                (sel16t, sel16_d), (negsel16t, negsel16_d), (selfselt, selfsel_d),
                (sel2t, sel2_d), (negsel2t, negsel2_d), (selfsel2t, selfsel2_d),
                (trit, tri_d), (ones8t, ones8_d), (ones128t, ones128_d),
                (poolidxt, poolidx_d), (maskpt, maskp_d), (placet, place_d),
                (cntt, cnt_d), (W1f, W1_d), (b1t, b1_d), (wlct, wlc_d),
                (blint, blin_d), (b2t, b2_d), (wlrt, wlr_d),
                (gidx_sb, gidx_d), (eidx_sb, eidx_d),
            ):
                nc.sync.dma_start(out=t[:], in_=d[:, :])
            nc.vector.memset(zerot16[:], 0.0)
            # zero-init the table: unselected rows must stay finite (0*NaN
            # would poison the PSUM-accumulating selector matmuls)
            nc.scalar.activation(
                out=table[:], in_=zerot16[:].to_broadcast([128, NPC]), func=AF.Copy
            )

            ddram = dram.tile([1, NPC], f16)
            yag_in = dram.tile([F, NPC], f16)
            yag_out = dram.tile([NC, F * NPC], f16)
            zag_in = dram.tile([1, NPC], f16)
            zag_out = dram.tile([NC, NPC], f16)
            qdram = dram.tile([1, NPC], f16)
            par_in = dram.tile([1, B], f32)
            par_out = dram.tile([NC, B], f32)

            # ---- phase A: deg -> dinv (block layout) -> node-ordered DRAM
            nc.sync.dma_start(out=gbuf[:, : ROWL * DPA], in_=w2a_d[:, :])
            nc.vector.tensor_reduce(
                out=dega[:], in_=gbuf[:, : ROWL * DPA].rearrange("p (c d) -> p c d", d=DPA),
                axis=AX, op=OP.add,
            )
            nc.sync.dma_start(out=gbuf[:, : ROWL * DPB], in_=w2b_d[:, :])
            nc.vector.tensor_reduce(
                out=degb[:], in_=gbuf[:, : ROWL * DPB].rearrange("p (c d) -> p c d", d=DPB),
                axis=AX, op=OP.add,
            )
            nc.vector.tensor_tensor(out=dega[:], in0=dega[:], in1=degb[:], op=OP.add)
            nc.scalar.activation(out=dega[:], in_=dega[:], func=AF.Sqrt)
            with nc.allow_low_precision(reason="dinv in fp16 is within tolerance"):
                nc.vector.reciprocal(out=dinvt[:], in_=dega[:])
            nc.sync.dma_start(
                out=ddram[:].rearrange("o (p c) -> (o p) c", c=ROWL), in_=dinvt[:]
            )

            # ---- y^T: xT quarters staged in gbuf(f32), dinv bcast in dinvw
            QS = NPC // 4  # 3136
            for q in range(4):
                nc.sync.dma_start(out=gbuf[0:16, :QS], in_=xT_d[:, q * QS : (q + 1) * QS])
                nc.sync.dma_start(
                    out=dinvw[0:F, :QS],
                    in_=ddram[0:1, q * QS : (q + 1) * QS].to_broadcast([F, QS]),
                )
                nc.vector.tensor_tensor(
                    out=msgsB[0:F, :QS], in0=gbuf[0:F, :QS], in1=dinvw[0:F, :QS],
                    op=OP.mult,
                )
                nc.sync.dma_start(
                    out=yag_in[:, q * QS : (q + 1) * QS], in_=msgsB[0:F, :QS]
                )
            nc.gpsimd.collective_compute(
                "AllGather", mybir.AluOpType.bypass, replica_groups=rg,
                ins=[yag_in[:]], outs=[yag_out[:]],
            )
            yag_v = yag_out[:].rearrange("k (f n) -> k f n", f=F)
            for k in range(NC):
                nc.sync.dma_start(out=table[16 * k : 16 * k + F, :], in_=yag_v[k])

            # ---- weights: W1 fp16; wz = W2 @ Wlin fp16; constants
            nc.scalar.copy(out=W1t[:], in_=W1f[:])
            with tc.tile_pool(name="pha", bufs=1) as pha, \
                 tc.tile_pool(name="pstA", bufs=1, space="PSUM") as pstA:
                ident = pha.tile([128, 128], f32)
                make_identity(nc, ident[:])
                W2t = pha.tile([128, 128], f32)
                nc.sync.dma_start(out=W2t[:], in_=W2_d[:, :])
                w2tp = pstA.tile([128, 512], f32)
                nc.tensor.transpose(out=w2tp[:, :128], in_=W2t[:], identity=ident[:])
                w2ts = pha.tile([128, 128], f32)
                nc.scalar.copy(out=w2ts[:], in_=w2tp[:, :128])
                wzp = pstA.tile([128, 1], f32)
                nc.tensor.matmul(out=wzp[:], lhsT=w2ts[:], rhs=wlct[:], start=True, stop=True)
                nc.scalar.copy(out=wzt[:], in_=wzp[:])
            nc.vector.tensor_copy(out=cntf[:], in_=cntt[:])
            nc.vector.reciprocal(out=rec[:], in_=cntf[:])
            nc.vector.tensor_tensor(out=cb[:], in0=b2t[:], in1=wlrt[:], op=OP.mult)
            nc.vector.tensor_reduce(out=cs[:], in_=cb[:], axis=AX, op=OP.add)
            nc.vector.tensor_tensor(out=cs[:], in0=cs[:], in1=blint[:], op=OP.add)

            # ---- wrep f32 -> fp16 via gbuf staging
            for wdw in range(NW):
                nc.sync.dma_start(
                    out=gbuf[:, : WLEN[wdw]],
                    in_=wrep_d[:, WOFF[wdw] : WOFF[wdw] + WLEN[wdw]],
                )
                nc.scalar.activation(
                    out=wrep16[:, WOFF[wdw] : WOFF[wdw] + WLEN[wdw]],
                    in_=gbuf[:, : WLEN[wdw]], func=AF.Copy,
                )

            # ---- conv layers
            conv_ctx = ExitStack()
            psc = conv_ctx.enter_context(tc.tile_pool(name="psc", bufs=1, space="PSUM"))
            pTpool = conv_ctx.enter_context(tc.tile_pool(name="pTp", bufs=1))
            zqpool = conv_ctx.enter_context(tc.tile_pool(name="zqp", bufs=1))
            epool = conv_ctx.enter_context(tc.tile_pool(name="ep", bufs=1))

            g16v = gbuf[:].bitcast(f16).rearrange("p (e two) -> p e two", two=2)

            def load_dinvw(wdw):
                span = CPWS[wdw] * M
                nc.sync.dma_start(
                    out=dinvw[0:F, :span],
                    in_=ddram[0:1, CST[wdw] * M : CST[wdw] * M + span].to_broadcast([F, span]),
                )

            def window_head(msgs, wdw):
                wl = WLEN[wdw]
                half = wl // 2  # cpw * C1
                nc.gpsimd.ap_gather(
                    out_ap=gbuf[:, :wl], in_ap=table[:].bitcast(f32),
                    idxs_ap=gidx_sb[:, WOFF[wdw] // 16 : (WOFF[wdw] + wl) // 16],
                    channels=128, num_elems=NPC // 2, d=1, num_idxs=wl,
                )
                nc.vector.tensor_tensor(
                    out=msgs[:, 0:half], in0=g16v[:, 0:half, 0],
                    in1=wrep16[:, WOFF[wdw] : WOFF[wdw] + half], op=OP.mult,
                )
                nc.vector.tensor_tensor(
                    out=msgs[:, half:wl], in0=g16v[:, half:wl, 1],
                    in1=wrep16[:, WOFF[wdw] + half : WOFF[wdw] + wl], op=OP.mult,
                )
                nc.vector.tensor_tensor_scan(
                    out=msgs[:, :wl], data0=msgs[:, :wl],
                    data1=zerot16[:].to_broadcast([128, wl]),
                    initial=0.0, op0=OP.add, op1=OP.add,
                )

            def extract_chunk(msgs, wdw, jl):
                """Two indirect_copies (even/odd subcell) -> E [128, 2*EBC]."""
                jj = CST[wdw] + jl
                cpw = CPWS[wdw]
                E = epool.tile([128, 2 * EBC], f16, tag=f"E{jj % 2}", bufs=1, name=f"E{jj}")
                for par in range(2):
                    base = (par * cpw + jl) * C1
                    nc.gpsimd.indirect_copy(
                        out=E[:, par * EBC : par * EBC + EBC],
                        data=msgs[:, base : base + C1],
                        idxs=eidx_sb[:, (jj * 2 + par) * EW : (jj * 2 + par + 1) * EW],
                        i_know_ap_gather_is_preferred=True,
                    )
                return E

            def merge_pm(pm, E, lhsp, lhsn, lhss, nsel, h, nsl):
                nc.tensor.matmul(
                    out=pm, lhsT=lhsp[:, 0:nsel],
                    rhs=E[:, 1 + h * 448 : 1 + h * 448 + 448],
                    start=True, stop=False,
                )
                nc.tensor.matmul(
                    out=pm, lhsT=lhsn[:, 0:nsel],
                    rhs=E[:, h * 448 : h * 448 + 448],
                    start=False, stop=False,
                )
                nc.tensor.matmul(
                    out=pm, lhsT=lhsp[:, 0:nsel],
                    rhs=E[:, EBC + 1 + h * 448 : EBC + 1 + h * 448 + 448],
                    start=False, stop=False,
                )
                nc.tensor.matmul(
                    out=pm, lhsT=lhsn[:, 0:nsel],
                    rhs=E[:, EBC + h * 448 : EBC + h * 448 + 448],
                    start=False, stop=False,
                )
                nc.tensor.matmul(
                    out=pm, lhsT=lhss[:, 0:nsel],
                    rhs=table[:, nsl], start=False, stop=True,
                )

            def l1_tail(msgs, wdw):
                for jl in range(CPWS[wdw]):
                    jj = CST[wdw] + jl
                    E = extract_chunk(msgs, wdw, jl)
                    zq = zqpool.tile([1, M], f16, tag=f"zq{jj % 2}", bufs=1, name=f"zq1_{jj}")
                    for h in range(2):
                        nsl = slice(jj * M + h * 448, jj * M + (h + 1) * 448)
                        wsl = slice(jl * M + h * 448, jl * M + (h + 1) * 448)
                        pm = psc.tile([16, 448], f32, tag=f"pm{h}", bufs=1,
                                      name=f"pm{wdw}_{jl}_{h}")
                        merge_pm(pm[0:F, :], E, sel16t, negsel16t, selfselt, F, h, nsl)
                        pT = pTpool.tile([16, 448], f16, tag=f"pT{h}", bufs=1,
                                         name=f"pT{wdw}_{jl}_{h}")
                        nc.vector.tensor_tensor(
                            out=pT[0:F, :], in0=pm[0:F, :], in1=dinvw[0:F, wsl],
                            op=OP.mult,
                        )
                        st = psc.tile([128, 448], f32, tag=f"st{h}", bufs=1,
                                      name=f"st{wdw}_{jl}_{h}")
                        nc.tensor.matmul(
                            out=st[:], lhsT=W1t[:], rhs=pT[0:F, :],
                            start=True, stop=True,
                        )
                        ht = pTpool.tile([128, 448], f16, tag=f"ht{h}", bufs=1,
                                         name=f"ht{wdw}_{jl}_{h}")
                        nc.scalar.activation(out=ht[:], in_=st[:], func=AF.Relu, bias=b1t[:])
                        qz = psc.tile([1, 448], f32, tag=f"qz{h}", bufs=1,
                                      name=f"qz{wdw}_{jl}_{h}")
                        nc.tensor.matmul(
                            out=qz[:], lhsT=wzt[:], rhs=ht[:], start=True, stop=True,
                        )
                        nc.vector.tensor_tensor(
                            out=zq[0:1, h * 448 : (h + 1) * 448], in0=qz[:],
                            in1=dinvw[0:1, wsl], op=OP.mult,
                        )
                    nc.sync.dma_start(
                        out=zag_in[0:1, jj * M : (jj + 1) * M], in_=zq[:]
                    )

            def l2_tail(msgs, wdw):
                for jl in range(CPWS[wdw]):
                    jj = CST[wdw] + jl
                    E = extract_chunk(msgs, wdw, jl)
                    zq = zqpool.tile([1, M], f16, tag=f"zq{jj % 2}", bufs=1, name=f"zq2_{jj}")
                    for h in range(2):
                        nsl = slice(jj * M + h * 448, jj * M + (h + 1) * 448)
                        wsl = slice(jl * M + h * 448, jl * M + (h + 1) * 448)
                        qm = psc.tile([1, 448], f32, tag=f"qz{h}", bufs=1,
                                      name=f"qm{wdw}_{jl}_{h}")
                        merge_pm(qm[:], E, sel2t, negsel2t, selfsel2t, 1, h, nsl)
                        nc.vector.tensor_tensor(
                            out=zq[0:1, h * 448 : (h + 1) * 448], in0=qm[:],
                            in1=dinvw[0:1, wsl], op=OP.mult,
                        )
                    nc.sync.dma_start(
                        out=qdram[0:1, jj * M : (jj + 1) * M], in_=zq[:]
                    )

            def layer(tail):
                load_dinvw(0)
                window_head(msgsA, 0)
                window_head(msgsB, 1)
                tail(msgsA, 0)
                load_dinvw(1)
                window_head(msgsA, 2)
                tail(msgsB, 1)
                load_dinvw(2)
                tail(msgsA, 2)

            layer(l1_tail)

            nc.gpsimd.collective_compute(
                "AllGather", mybir.AluOpType.bypass, replica_groups=rg,
                ins=[zag_in[:]], outs=[zag_out[:]],
            )
            for k in range(NC):
                nc.sync.dma_start(
                    out=table[16 * k + 10 : 16 * k + 11, :], in_=zag_out[k : k + 1, :]
                )

            layer(l2_tail)
            conv_ctx.close()

            # ---- pooling
            nc.sync.dma_start(
                out=qblk[:], in_=qdram[:].rearrange("o (p c) -> (o p) c", c=ROWL)
            )
            nc.vector.tensor_tensor_scan(
                out=qP[:], data0=qblk[:],
                data1=zerot16[:].to_broadcast([128, ROWL]),
                initial=0.0, op0=OP.add, op1=OP.add,
            )
            with tc.tile_pool(name="pst2", bufs=1, space="PSUM") as pst2:
                offp = pst2.tile([128, 1], f32)
                nc.tensor.matmul(
                    out=offp[:], lhsT=trit[:], rhs=qP[:, ROWL - 1 : ROWL],
                    start=True, stop=True,
                )
                nc.scalar.copy(out=offs[:], in_=offp[:])
                nc.gpsimd.ap_gather(
                    out_ap=ext[:], in_ap=qP[:], idxs_ap=poolidxt[:],
                    channels=128, num_elems=ROWL, d=1, num_idxs=NPOOL,
                )
                nc.vector.scalar_tensor_tensor(
                    out=masked[:], in0=ext[:], scalar=offs[:], in1=maskpt[:],
                    op0=OP.add, op1=OP.mult,
                )
                epp = pst2.tile([1, NPOOL], f32)
                nc.tensor.matmul(
                    out=epp[:], lhsT=ones128t[:], rhs=masked[:], start=True, stop=True,
                )
                nc.scalar.copy(out=Eps[:], in_=epp[:])
                nc.vector.memset(Pp[:], 0.0)
                nc.vector.tensor_copy(out=Pp[0:1, 0:1], in_=Eps[0:1, 0:1])
                nc.vector.tensor_tensor(
                    out=Pp[0:1, 1:NPOOL], in0=Eps[0:1, 1:NPOOL],
                    in1=Eps[0:1, 0 : NPOOL - 1], op=OP.subtract,
                )
                nc.gpsimd.ap_gather(
                    out_ap=placed[:], in_ap=Pp[:], idxs_ap=placet[:],
                    channels=16, num_elems=NPOOL + 1, d=1, num_idxs=B,
                )
                nc.sync.dma_start(out=par_in[:], in_=placed[0:1, :])
                nc.gpsimd.collective_compute(
                    "AllGather", mybir.AluOpType.bypass, replica_groups=rg,
                    ins=[par_in[:]], outs=[par_out[:]],
                )
                nc.sync.dma_start(out=arp[:], in_=par_out[:, :])
                mrg = pst2.tile([1, B], f32)
                nc.tensor.matmul(
                    out=mrg[:], lhsT=ones8t[:], rhs=arp[:], start=True, stop=True,
                )
                nc.vector.tensor_tensor(out=res[:], in0=mrg[:], in1=rec[:], op=OP.mult)
                nc.vector.tensor_tensor(
                    out=res[:], in0=res[:], in1=cs[:].to_broadcast([1, B]), op=OP.add
                )
                nc.sync.dma_start(out=out_d[:, :], in_=res[:])

    nc.compile()
    return nc


_CACHE = {}


def kernel(**inputs):
    from concourse.bass_utils import run_bass_kernel_spmd

    cores, meta = prep(
        inputs["x"], inputs["edge_index"], inputs["edge_weight"], inputs["batch"]
    )
    key = (meta["C1"], meta["DP"])
    if key not in _CACHE:
        _CACHE[key] = build_program(*key)
    nc = _CACHE[key]

    W1 = np.asarray(inputs["W1"], dtype=np.float32)
    b1 = np.asarray(inputs["b1"], dtype=np.float32).reshape(128, 1)
    W2 = np.asarray(inputs["W2"], dtype=np.float32)
    wlc = np.asarray(inputs["Wlin"], dtype=np.float32).reshape(128, 1)
    wlr = np.asarray(inputs["Wlin"], dtype=np.float32).reshape(1, 128)
    blin = np.asarray(inputs["blin"], dtype=np.float32).reshape(1, 1)
    b2r = np.asarray(inputs["b2"], dtype=np.float32).reshape(1, 128)

    in_maps = []
    for c in range(NC):
        cr = dict(cores[c])
        cr.update(W1=W1, b1=b1, W2=W2, wlin_col=wlc, wlin_row=wlr, blin=blin, b2row=b2r)
        in_maps.append(cr)
    res = run_bass_kernel_spmd(nc, in_maps, list(range(NC)))
    out = np.asarray(res.results[0]["out"], dtype=np.float32).reshape(B, 1)
    return out


# revision 13
# speedup vs baseline: 1.5994x; 1.0919x over previous
"""GCN (2-layer GCNConv + mean-pool + linear) on 8 Trainium2 NeuronCores.

Strategy (v2: fp16 edge pipeline, static parity subcells, pair-gathers):
  - dst-shard nodes across 8 cores (12544 each); self-loops REMOVED from edge
    lists (handled as a PSUM-accumulated matmul term against the feature
    table, selected by a per-core 0/1 matrix).
  - edges bucketed into static cells (src-chunk group k, dst chunk j, src
    parity) of C1 slots, dst-sorted within a cell; chunks processed in 3
    windows of 5/5/4; the window stream is [even subcells | odd subcells].
  - ap_gather moves 4-byte units, so the fp16 feature table [128, 12544]
    (group k rows 16k+f hold y^T[f] = dinv*x; row 16k+10 holds layer-2's z')
    is gathered through its f32 bitcast with idx = src//2; strided fp16
    multiplies by the edge weights select the parity half and apply w ->
    in-place prefix scan over the window -> per-(chunk,parity) indirect_copy
    (<=1024 idxs per call: walrus ISA limit) extracts per-node boundary
    prefixes -> per-chunk merge via +/- selector matmuls accumulated in PSUM
    together with the self-loop term -> * dinv_dst -> W1 + relu + z (layer 1).
  - layer 2 propagates one scalar per node (z = h1 @ (W2 @ Wlin)); mean pool
    via a DRAM roundtrip into a [128, 98] block layout, per-partition scan +
    triangular-matmul offsets, tiny ap_gather of graph ends, masked
    partition-collapse matmul, and AllGather + ones-matmul instead of an
    AllReduce.
All floating-point math runs on device; the host only sorts/permutes indices,
pads with zeros/ones, and builds constant 0/+-1 selector matrices.
"""
import numpy as np

N = 100000
NC = 8
NPC = 12544
B = 256
F = 10
S = 14
M = NPC // S          # 896
ROWL = NPC // 128     # 98
NW = 3
CST = [0, 5, 10]      # window chunk starts
CPWS = [5, 5, 4]      # chunks per window
NPOOL = 64
EBC = 928             # boundary slots per (chunk, parity): 4B-aligned idx slices
EW = EBC // 16        # 58


def _ceil16(v):
    return ((v + 15) // 16) * 16


def _wrap16(vals):
    v = np.asarray(vals)
    assert len(v) % 16 == 0
    return v.reshape(len(v) // 16, 16).T.copy()


def prep(x, edge_index, edge_weight, batch):
    """Pure index/permutation prep. Returns (per-core input dicts, meta)."""
    src = np.asarray(edge_index[0], dtype=np.int64)
    dst = np.asarray(edge_index[1], dtype=np.int64)
    w = np.asarray(edge_weight, dtype=np.float32)
    batch = np.asarray(batch, dtype=np.int64)
    x = np.asarray(x, dtype=np.float32)

    DP = 1 + int(np.bincount(dst, minlength=N).max())

    core_e = dst // NPC
    g_all = src // NPC
    dstloc = dst - core_e * NPC
    chunk_all = dstloc // M
    par_all = src % 2
    cell = ((core_e * NC + g_all) * S + chunk_all) * 2 + par_all
    C1 = _ceil16(int(np.bincount(cell, minlength=NC * NC * S * 2).max()) + 1)
    TSL = S * 2 * C1              # total stream slots per group
    WOFF = [0, 10 * C1, 20 * C1]  # window stream offsets
    GL = TSL // 16

    cnt = np.maximum(np.bincount(batch, minlength=B), 1).astype(np.int32)

    sel16 = np.zeros((128, 16), dtype=np.float16)
    sel16[np.arange(128), np.arange(128) % 16] = 1.0
    negsel16 = (-sel16).astype(np.float16)
    sel2 = np.zeros((128, 1), dtype=np.float16)
    sel2[10::16] = 1.0
    negsel2 = (-sel2).astype(np.float16)
    tri = np.zeros((128, 128), dtype=np.float32)
    tri[np.triu_indices(128, 1)] = 1.0
    ones8 = np.ones((8, 1), dtype=np.float32)
    ones128 = np.ones((128, 1), dtype=np.float32)

    cores = []
    for c in range(NC):
        lo = c * NPC
        hi = min((c + 1) * NPC, N)
        nreal = hi - lo

        xT = np.zeros((16, NPC), dtype=np.float32)
        xT[:F, :nreal] = x[lo:hi].T

        # weighted-degree pad: node n -> (p=n//98, c2=n%98), slot 0 = self w=1
        w2 = np.zeros((128, ROWL, DP), dtype=np.float32)
        w2[:, :, 0] = 1.0
        emask = (dst >= lo) & (dst < hi)
        es, ed, ew = src[emask], (dst[emask] - lo), w[emask]
        od = np.argsort(ed, kind="stable")
        ed_s, ew_s = ed[od], ew[od]
        starts = np.searchsorted(ed_s, np.arange(NPC))
        rank = np.arange(len(ed_s)) - starts[ed_s]
        w2[ed_s // ROWL, ed_s % ROWL, 1 + rank] = ew_s
        DPA = (DP + 1) // 2
        w2a = w2[:, :, :DPA].reshape(128, ROWL * DPA).copy()
        w2b = w2[:, :, DPA:].reshape(128, ROWL * (DP - DPA)).copy()

        eg = es // NPC
        esl = es - eg * NPC
        echunk = ed // M
        epar = esl % 2
        gidx = np.zeros((128, GL), dtype=np.int16)
        wrep = np.zeros((128, TSL), dtype=np.float32)
        eidx = np.zeros((128, S * 2 * EW), dtype=np.uint16)
        for k in range(NC):
            idx_slots = np.zeros(TSL, dtype=np.int16)
            w_slots = np.zeros(TSL, dtype=np.float32)
            for wdw in range(NW):
                cpw = CPWS[wdw]
                for jl in range(cpw):
                    j = CST[wdw] + jl
                    for par in range(2):
                        m = (eg == k) & (echunk == j) & (epar == par)
                        dk, wk, sk = ed[m], ew[m], esl[m] // 2
                        o = np.argsort(dk, kind="stable")
                        dk, wk, sk = dk[o], wk[o], sk[o]
                        n_e = len(dk)
                        assert n_e + 1 <= C1
                        base = WOFF[wdw] + (par * cpw + jl) * C1
                        idx_slots[base + 1 : base + 1 + n_e] = sk
                        w_slots[base + 1 : base + 1 + n_e] = wk
                        bounds = np.zeros(EBC, dtype=np.uint16)
                        bounds[: M + 1] = np.searchsorted(
                            dk, np.arange(j * M, j * M + M + 1)
                        ).astype(np.uint16)
                        eidx[16 * k : 16 * (k + 1),
                             (j * 2 + par) * EW : (j * 2 + par + 1) * EW] = _wrap16(bounds)
            gidx[16 * k : 16 * (k + 1), :] = _wrap16(idx_slots)
            wrep[16 * k : 16 * (k + 1), :] = w_slots[None, :]

        selfsel = np.zeros((128, 16), dtype=np.float16)
        selfsel[16 * c + np.arange(16), np.arange(16)] = 1.0
        selfsel2 = np.zeros((128, 1), dtype=np.float16)
        selfsel2[16 * c + 10] = 1.0

        gmin, gmax = int(batch[lo]), int(batch[hi - 1])
        glist = np.arange(gmin, gmax + 1)
        n_gc = len(glist)
        assert n_gc <= NPOOL
        gends = np.minimum(np.searchsorted(batch, glist, side="right") - lo, NPC)
        ends_node = np.maximum(gends - 1, 0)
        p_i = (ends_node // ROWL).astype(np.int64)
        c_i = (ends_node % ROWL).astype(np.int16)
        vals_by_group = np.zeros((NC, NPOOL), dtype=np.int16)
        vals_by_group[p_i // 16, np.arange(n_gc)] = c_i
        poolidx = np.zeros((128, NPOOL // 16), dtype=np.int16)
        for G in range(NC):
            poolidx[16 * G : 16 * (G + 1), :] = _wrap16(vals_by_group[G])
        maskp = np.zeros((128, NPOOL), dtype=np.float32)
        maskp[p_i, np.arange(n_gc)] = 1.0
        place_vals = np.full(B, NPOOL, dtype=np.int16)
        place_vals[gmin : gmax + 1] = np.arange(n_gc, dtype=np.int16)
        place = _wrap16(place_vals)

        cores.append(
            dict(
                xT=xT, w2a=w2a, w2b=w2b, gidx=gidx, wrep=wrep, eidx=eidx,
                sel16=sel16, negsel16=negsel16, selfsel=selfsel,
                sel2=sel2, negsel2=negsel2, selfsel2=selfsel2,
                tri=tri, ones8=ones8, ones128=ones128,
                poolidx=poolidx, maskp=maskp, place=place,
                cnt=cnt.reshape(1, B),
            )
        )
    return cores, dict(C1=C1, DP=DP)


# ------------------------------------------------------------------ device
def build_program(C1, DP):
    import concourse.bass as bass
    import concourse.bacc as bacc
    import concourse.mybir as mybir
    import concourse.tile as tile
    from concourse.masks import make_identity

    f32 = mybir.dt.float32
    f16 = mybir.dt.float16
    i16 = mybir.dt.int16
    u16 = mybir.dt.uint16
    i32 = mybir.dt.int32
    AX = mybir.AxisListType.X
    OP = mybir.AluOpType
    AF = mybir.ActivationFunctionType

    TSL = S * 2 * C1
    WOFF = [0, 10 * C1, 20 * C1]
    WLEN = [10 * C1, 10 * C1, 8 * C1]
    GL = TSL // 16
    DPA = (DP + 1) // 2
    DPB = DP - DPA

    nc = bacc.Bacc("TRN2", target_bir_lowering=False, debug=False, num_devices=NC)

    def din(name, shape, dt=f32):
        return nc.dram_tensor(name, shape, dt, kind="ExternalInput")

    xT_d = din("xT", [16, NPC])
    w2a_d = din("w2a", [128, ROWL * DPA])
    w2b_d = din("w2b", [128, ROWL * DPB])
    gidx_d = din("gidx", [128, GL], i16)
    wrep_d = din("wrep", [128, TSL])
    eidx_d = din("eidx", [128, S * 2 * EW], u16)
    sel16_d = din("sel16", [128, 16], f16)
    negsel16_d = din("negsel16", [128, 16], f16)
    selfsel_d = din("selfsel", [128, 16], f16)
    sel2_d = din("sel2", [128, 1], f16)
    negsel2_d = din("negsel2", [128, 1], f16)
    selfsel2_d = din("selfsel2", [128, 1], f16)
    tri_d = din("tri", [128, 128])
    ones8_d = din("ones8", [8, 1])
    ones128_d = din("ones128", [128, 1])
    poolidx_d = din("poolidx", [128, NPOOL // 16], i16)
    maskp_d = din("maskp", [128, NPOOL])
    place_d = din("place", [16, 16], i16)
    cnt_d = din("cnt", [1, B], i32)
    W1_d = din("W1", [F, 128])
    b1_d = din("b1", [128, 1])
    W2_d = din("W2", [128, 128])
    wlc_d = din("wlin_col", [128, 1])
    blin_d = din("blin", [1, 1])
    b2_d = din("b2row", [1, 128])
    wlr_d = din("wlin_row", [1, 128])
    out_d = nc.dram_tensor("out", [1, B], f32, kind="ExternalOutput")

    rg = [list(range(NC))]

    with tile.TileContext(nc) as tc:
        from contextlib import ExitStack

        with ExitStack() as ctx:
            sb = ctx.enter_context(tc.tile_pool(name="sb", bufs=1))
            big = ctx.enter_context(tc.tile_pool(name="big", bufs=1))
            dram = ctx.enter_context(tc.tile_pool(name="dram", bufs=1, space="DRAM"))

            # ---- resident tiles (bigA/bigB hold gathered f32 pairs, then the
            # in-place compacted fp16 stream, then the in-place scan)
            bigA = big.tile([128, 20 * C1], f16)
            bigB = big.tile([128, 20 * C1], f16)
            wrep16 = big.tile([128, TSL], f16)
            table = big.tile([128, NPC], f16)
            dinvw = big.tile([16, NPC // 2], f16)
            gidx_sb = big.tile([128, GL], i16)
            eidx_sb = big.tile([128, S * 2 * EW], u16)

            # ---- small constants / scratch
            sel16t = sb.tile([128, 16], f16)
            negsel16t = sb.tile([128, 16], f16)
            selfselt = sb.tile([128, 16], f16)
            sel2t = sb.tile([128, 1], f16)
            negsel2t = sb.tile([128, 1], f16)
            selfsel2t = sb.tile([128, 1], f16)
            trit = sb.tile([128, 128], f32)
            ones8t = sb.tile([8, 1], f32)
            ones128t = sb.tile([128, 1], f32)
            poolidxt = sb.tile([128, NPOOL // 16], i16)
            maskpt = sb.tile([128, NPOOL], f32)
            placet = sb.tile([16, 16], i16)
            cntt = sb.tile([1, B], i32)
            W1f = sb.tile([F, 128], f32)
            W1t = sb.tile([F, 128], f16)
            b1t = sb.tile([128, 1], f32)
            wlct = sb.tile([128, 1], f32)
            blint = sb.tile([1, 1], f32)
            b2t = sb.tile([1, 128], f32)
            wlrt = sb.tile([1, 128], f32)
            wzt = sb.tile([128, 1], f16)
            zerot16 = sb.tile([128, 1], f16)
            dega = sb.tile([128, ROWL], f32)
            degb = sb.tile([128, ROWL], f32)
            dinvt = sb.tile([128, ROWL], f16)
            qblk = sb.tile([128, ROWL], f16)
            qP = sb.tile([128, ROWL], f32)
            offs = sb.tile([128, 1], f32)
            ext = sb.tile([128, NPOOL], f32)
            masked = sb.tile([128, NPOOL], f32)
            Eps = sb.tile([1, NPOOL], f32)
            Pp = sb.tile([16, NPOOL + 1], f32)
            placed = sb.tile([16, B], f32)
            arp = sb.tile([8, B], f32)
            res = sb.tile([1, B], f32)
            cntf = sb.tile([1, B], f32)
            rec = sb.tile([1, B], f32)
            cb = sb.tile([1, 128], f32)
            cs = sb.tile([1, 1], f32)

            for t, d in (
                (sel16t, sel16_d), (negsel16t, negsel16_d), (selfselt, selfsel_d),
                (sel2t, sel2_d), (negsel2t, negsel2_d), (selfsel2t, selfsel2_d),
                (trit, tri_d), (ones8t, ones8_d), (ones128t, ones128_d),
                (poolidxt, poolidx_d), (maskpt, maskp_d), (placet, place_d),
                (cntt, cnt_d), (W1f, W1_d), (b1t, b1_d), (wlct, wlc_d),
                (blint, blin_d), (b2t, b2_d), (wlrt, wlr_d),
                (gidx_sb, gidx_d), (eidx_sb, eidx_d),
            ):
                nc.sync.dma_start(out=t[:], in_=d[:, :])
            nc.vector.memset(zerot16[:], 0.0)
            # zero-init the table: unselected rows must stay finite (0*NaN
            # would poison the PSUM-accumulating selector matmuls)
            nc.scalar.activation(
                out=table[:], in_=zerot16[:].to_broadcast([128, NPC]), func=AF.Copy
            )

            ddram = dram.tile([1, NPC], f16)
            yag_in = dram.tile([F, NPC], f16)
            yag_out = dram.tile([NC, F * NPC], f16)
            zag_in = dram.tile([1, NPC], f16)
            zag_out = dram.tile([NC, NPC], f16)
            qdram = dram.tile([1, NPC], f16)
            par_in = dram.tile([1, B], f32)
            par_out = dram.tile([NC, B], f32)

            # ---- phase A: deg -> dinv (block layout) -> node-ordered DRAM
            bAf = bigA[:].bitcast(f32)
            bBf = bigB[:].bitcast(f32)
            nc.sync.dma_start(out=bAf[:, : ROWL * DPA], in_=w2a_d[:, :])
            nc.sync.dma_start(out=bBf[:, : ROWL * DPB], in_=w2b_d[:, :])
            nc.vector.tensor_reduce(
                out=dega[:], in_=bAf[:, : ROWL * DPA].rearrange("p (c d) -> p c d", d=DPA),
                axis=AX, op=OP.add,
            )
            nc.vector.tensor_reduce(
                out=degb[:], in_=bBf[:, : ROWL * DPB].rearrange("p (c d) -> p c d", d=DPB),
                axis=AX, op=OP.add,
            )
            nc.vector.tensor_tensor(out=dega[:], in0=dega[:], in1=degb[:], op=OP.add)
            nc.scalar.activation(out=dega[:], in_=dega[:], func=AF.Sqrt)
            with nc.allow_low_precision(reason="dinv in fp16 is within tolerance"):
                nc.vector.reciprocal(out=dinvt[:], in_=dega[:])
            nc.sync.dma_start(
                out=ddram[:].rearrange("o (p c) -> (o p) c", c=ROWL), in_=dinvt[:]
            )

            # ---- y^T: xT halves staged in bigA(f32), dinv bcast in dinvw
            HS = NPC // 2  # 6272
            for q in range(2):
                nc.sync.dma_start(out=bAf[0:16, :HS], in_=xT_d[:, q * HS : (q + 1) * HS])
                nc.sync.dma_start(
                    out=dinvw[0:F, :HS],
                    in_=ddram[0:1, q * HS : (q + 1) * HS].to_broadcast([F, HS]),
                )
                nc.vector.tensor_tensor(
                    out=bigB[0:F, q * HS : (q + 1) * HS], in0=bAf[0:F, :HS],
                    in1=dinvw[0:F, :HS], op=OP.mult,
                )
                nc.sync.dma_start(
                    out=yag_in[:, q * HS : (q + 1) * HS],
                    in_=bigB[0:F, q * HS : (q + 1) * HS],
                )

            # ---- weights: W1 fp16; wz = W2 @ Wlin fp16; constants
            nc.scalar.copy(out=W1t[:], in_=W1f[:])
            with tc.tile_pool(name="pha", bufs=1) as pha, \
                 tc.tile_pool(name="pstA", bufs=1, space="PSUM") as pstA:
                ident = pha.tile([128, 128], f32)
                make_identity(nc, ident[:])
                W2t = pha.tile([128, 128], f32)
                nc.sync.dma_start(out=W2t[:], in_=W2_d[:, :])
                w2tp = pstA.tile([128, 512], f32)
                nc.tensor.transpose(out=w2tp[:, :128], in_=W2t[:], identity=ident[:])
                w2ts = pha.tile([128, 128], f32)
                nc.scalar.copy(out=w2ts[:], in_=w2tp[:, :128])
                wzp = pstA.tile([128, 1], f32)
                nc.tensor.matmul(out=wzp[:], lhsT=w2ts[:], rhs=wlct[:], start=True, stop=True)
                nc.scalar.copy(out=wzt[:], in_=wzp[:])
            nc.vector.tensor_copy(out=cntf[:], in_=cntt[:])
            nc.vector.reciprocal(out=rec[:], in_=cntf[:])
            nc.vector.tensor_tensor(out=cb[:], in0=b2t[:], in1=wlrt[:], op=OP.mult)
            nc.vector.tensor_reduce(out=cs[:], in_=cb[:], axis=AX, op=OP.add)
            nc.vector.tensor_tensor(out=cs[:], in0=cs[:], in1=blint[:], op=OP.add)

            # ---- wrep f32 -> fp16 via bigA/bigB staging (overlaps AllGather)
            for wdw in range(NW):
                stg = bAf if wdw % 2 == 0 else bBf
                nc.sync.dma_start(
                    out=stg[:, : WLEN[wdw]],
                    in_=wrep_d[:, WOFF[wdw] : WOFF[wdw] + WLEN[wdw]],
                )
                nc.scalar.activation(
                    out=wrep16[:, WOFF[wdw] : WOFF[wdw] + WLEN[wdw]],
                    in_=stg[:, : WLEN[wdw]], func=AF.Copy,
                )
            nc.gpsimd.collective_compute(
                "AllGather", mybir.AluOpType.bypass, replica_groups=rg,
                ins=[yag_in[:]], outs=[yag_out[:]],
            )
            yag_v = yag_out[:].rearrange("k (f n) -> k f n", f=F)
            for k in range(NC):
                nc.sync.dma_start(out=table[16 * k : 16 * k + F, :], in_=yag_v[k])

            # ---- conv layers
            conv_ctx = ExitStack()
            psc = conv_ctx.enter_context(tc.tile_pool(name="psc", bufs=1, space="PSUM"))
            pTpool = conv_ctx.enter_context(tc.tile_pool(name="pTp", bufs=1))
            zqpool = conv_ctx.enter_context(tc.tile_pool(name="zqp", bufs=1))
            epool = conv_ctx.enter_context(tc.tile_pool(name="ep", bufs=1))

            def load_dinvw(wdw):
                span = CPWS[wdw] * M
                nc.sync.dma_start(
                    out=dinvw[0:F, :span],
                    in_=ddram[0:1, CST[wdw] * M : CST[wdw] * M + span].to_broadcast([F, span]),
                )

            def window_head(msgs, wdw):
                wl = WLEN[wdw]
                half = wl // 2  # cpw * C1
                mf32 = msgs[:].bitcast(f32)
                mprs = msgs[:].rearrange("p (e two) -> p e two", two=2)
                nc.gpsimd.ap_gather(
                    out_ap=mf32[:, :wl], in_ap=table[:].bitcast(f32),
                    idxs_ap=gidx_sb[:, WOFF[wdw] // 16 : (WOFF[wdw] + wl) // 16],
                    channels=128, num_elems=NPC // 2, d=1, num_idxs=wl,
                )
                # in-place compaction: even srcs read pair slot 0 (writes trail
                # reads), odd srcs read slot 1 of the upper pair region (reads
                # stay ahead of writes)
                nc.vector.tensor_tensor(
                    out=msgs[:, 0:half], in0=mprs[:, 0:half, 0],
                    in1=wrep16[:, WOFF[wdw] : WOFF[wdw] + half], op=OP.mult,
                )
                nc.vector.tensor_tensor(
                    out=msgs[:, half:wl], in0=mprs[:, half:wl, 1],
                    in1=wrep16[:, WOFF[wdw] + half : WOFF[wdw] + wl], op=OP.mult,
                )
                nc.vector.tensor_tensor_scan(
                    out=msgs[:, :wl], data0=msgs[:, :wl],
                    data1=zerot16[:].to_broadcast([128, wl]),
                    initial=0.0, op0=OP.add, op1=OP.add,
                )

            def extract_chunk(msgs, wdw, jl):
                """Two indirect_copies (even/odd subcell) -> E [128, 2*EBC]."""
                jj = CST[wdw] + jl
                cpw = CPWS[wdw]
                E = epool.tile([128, 2 * EBC], f16, tag=f"E{jj % 2}", bufs=1, name=f"E{jj}")
                for par in range(2):
                    base = (par * cpw + jl) * C1
                    nc.gpsimd.indirect_copy(
                        out=E[:, par * EBC : par * EBC + EBC],
                        data=msgs[:, base : base + C1],
                        idxs=eidx_sb[:, (jj * 2 + par) * EW : (jj * 2 + par + 1) * EW],
                        i_know_ap_gather_is_preferred=True,
                    )
                return E

            def merge_pm(pm, E, lhsp, lhsn, lhss, nsel, h, nsl):
                nc.tensor.matmul(
                    out=pm, lhsT=lhsp[:, 0:nsel],
                    rhs=E[:, 1 + h * 448 : 1 + h * 448 + 448],
                    start=True, stop=False,
                )
                nc.tensor.matmul(
                    out=pm, lhsT=lhsn[:, 0:nsel],
                    rhs=E[:, h * 448 : h * 448 + 448],
                    start=False, stop=False,
                )
                nc.tensor.matmul(
                    out=pm, lhsT=lhsp[:, 0:nsel],
                    rhs=E[:, EBC + 1 + h * 448 : EBC + 1 + h * 448 + 448],
                    start=False, stop=False,
                )
                nc.tensor.matmul(
                    out=pm, lhsT=lhsn[:, 0:nsel],
                    rhs=E[:, EBC + h * 448 : EBC + h * 448 + 448],
                    start=False, stop=False,
                )
                nc.tensor.matmul(
                    out=pm, lhsT=lhss[:, 0:nsel],
                    rhs=table[:, nsl], start=False, stop=True,
                )

            def l1_tail(msgs, wdw):
                for jl in range(CPWS[wdw]):
                    jj = CST[wdw] + jl
                    E = extract_chunk(msgs, wdw, jl)
                    zq = zqpool.tile([1, M], f16, tag=f"zq{jj % 2}", bufs=1, name=f"zq1_{jj}")
                    for h in range(2):
                        nsl = slice(jj * M + h * 448, jj * M + (h + 1) * 448)
                        wsl = slice(jl * M + h * 448, jl * M + (h + 1) * 448)
                        pm = psc.tile([16, 448], f32, tag=f"pm{h}", bufs=1,
                                      name=f"pm{wdw}_{jl}_{h}")
                        merge_pm(pm[0:F, :], E, sel16t, negsel16t, selfselt, F, h, nsl)
                        pT = pTpool.tile([16, 448], f16, tag=f"pT{h}", bufs=1,
                                         name=f"pT{wdw}_{jl}_{h}")
                        nc.vector.tensor_tensor(
                            out=pT[0:F, :], in0=pm[0:F, :], in1=dinvw[0:F, wsl],
                            op=OP.mult,
                        )
                        st = psc.tile([128, 448], f32, tag=f"st{h}", bufs=1,
                                      name=f"st{wdw}_{jl}_{h}")
                        nc.tensor.matmul(
                            out=st[:], lhsT=W1t[:], rhs=pT[0:F, :],
                            start=True, stop=True,
                        )
                        ht = pTpool.tile([128, 448], f16, tag=f"ht{h}", bufs=1,
                                         name=f"ht{wdw}_{jl}_{h}")
                        nc.scalar.activation(out=ht[:], in_=st[:], func=AF.Relu, bias=b1t[:])
                        qz = psc.tile([1, 448], f32, tag=f"qz{h}", bufs=1,
                                      name=f"qz{wdw}_{jl}_{h}")
                        nc.tensor.matmul(
                            out=qz[:], lhsT=wzt[:], rhs=ht[:], start=True, stop=True,
                        )
                        nc.vector.tensor_tensor(
                            out=zq[0:1, h * 448 : (h + 1) * 448], in0=qz[:],
                            in1=dinvw[0:1, wsl], op=OP.mult,
                        )
                    nc.sync.dma_start(
                        out=zag_in[0:1, jj * M : (jj + 1) * M], in_=zq[:]
                    )

            def l2_tail(msgs, wdw):
                for jl in range(CPWS[wdw]):
                    jj = CST[wdw] + jl
                    E = extract_chunk(msgs, wdw, jl)
                    zq = zqpool.tile([1, M], f16, tag=f"zq{jj % 2}", bufs=1, name=f"zq2_{jj}")
                    for h in range(2):
                        nsl = slice(jj * M + h * 448, jj * M + (h + 1) * 448)
                        wsl = slice(jl * M + h * 448, jl * M + (h + 1) * 448)
                        qm = psc.tile([1, 448], f32, tag=f"qz{h}", bufs=1,
                                      name=f"qm{wdw}_{jl}_{h}")
                        merge_pm(qm[:], E, sel2t, negsel2t, selfsel2t, 1, h, nsl)
                        nc.vector.tensor_tensor(
                            out=zq[0:1, h * 448 : (h + 1) * 448], in0=qm[:],
                            in1=dinvw[0:1, wsl], op=OP.mult,
                        )
                    nc.sync.dma_start(
                        out=qdram[0:1, jj * M : (jj + 1) * M], in_=zq[:]
                    )

            def layer(tail):
                load_dinvw(0)
                window_head(bigA, 0)
                window_head(bigB, 1)
                tail(bigA, 0)
                load_dinvw(1)
                window_head(bigA, 2)
                tail(bigB, 1)
                load_dinvw(2)
                tail(bigA, 2)

            layer(l1_tail)

            nc.gpsimd.collective_compute(
                "AllGather", mybir.AluOpType.bypass, replica_groups=rg,
                ins=[zag_in[:]], outs=[zag_out[:]],
            )
            for k in range(NC):
                nc.sync.dma_start(
                    out=table[16 * k + 10 : 16 * k + 11, :], in_=zag_out[k : k + 1, :]
                )

            layer(l2_tail)
            conv_ctx.close()

            # ---- pooling
            nc.sync.dma_start(
                out=qblk[:], in_=qdram[:].rearrange("o (p c) -> (o p) c", c=ROWL)
            )
            nc.vector.tensor_tensor_scan(
                out=qP[:], data0=qblk[:],
                data1=zerot16[:].to_broadcast([128, ROWL]),
                initial=0.0, op0=OP.add, op1=OP.add,
            )
            with tc.tile_pool(name="pst2", bufs=1, space="PSUM") as pst2:
                offp = pst2.tile([128, 1], f32)
                nc.tensor.matmul(
                    out=offp[:], lhsT=trit[:], rhs=qP[:, ROWL - 1 : ROWL],
                    start=True, stop=True,
                )
                nc.scalar.copy(out=offs[:], in_=offp[:])
                nc.gpsimd.ap_gather(
                    out_ap=ext[:], in_ap=qP[:], idxs_ap=poolidxt[:],
                    channels=128, num_elems=ROWL, d=1, num_idxs=NPOOL,
                )
                nc.vector.scalar_tensor_tensor(
                    out=masked[:], in0=ext[:], scalar=offs[:], in1=maskpt[:],
                    op0=OP.add, op1=OP.mult,
                )
                epp = pst2.tile([1, NPOOL], f32)
                nc.tensor.matmul(
                    out=epp[:], lhsT=ones128t[:], rhs=masked[:], start=True, stop=True,
                )
                nc.scalar.copy(out=Eps[:], in_=epp[:])
                nc.vector.memset(Pp[:], 0.0)
                nc.vector.tensor_copy(out=Pp[0:1, 0:1], in_=Eps[0:1, 0:1])
                nc.vector.tensor_tensor(
                    out=Pp[0:1, 1:NPOOL], in0=Eps[0:1, 1:NPOOL],
                    in1=Eps[0:1, 0 : NPOOL - 1], op=OP.subtract,
                )
                nc.gpsimd.ap_gather(
                    out_ap=placed[:], in_ap=Pp[:], idxs_ap=placet[:],
                    channels=16, num_elems=NPOOL + 1, d=1, num_idxs=B,
                )
                nc.sync.dma_start(out=par_in[:], in_=placed[0:1, :])
                nc.gpsimd.collective_compute(
                    "AllGather", mybir.AluOpType.bypass, replica_groups=rg,
                    ins=[par_in[:]], outs=[par_out[:]],
                )
                nc.sync.dma_start(out=arp[:], in_=par_out[:, :])
                mrg = pst2.tile([1, B], f32)
                nc.tensor.matmul(
                    out=mrg[:], lhsT=ones8t[:], rhs=arp[:], start=True, stop=True,
                )
                nc.vector.tensor_tensor(out=res[:], in0=mrg[:], in1=rec[:], op=OP.mult)
                nc.vector.tensor_tensor(
                    out=res[:], in0=res[:], in1=cs[:].to_broadcast([1, B]), op=OP.add
                )
                nc.sync.dma_start(out=out_d[:, :], in_=res[:])

    nc.compile()
    return nc


_CACHE = {}


def kernel(**inputs):
    from concourse.bass_utils import run_bass_kernel_spmd

    cores, meta = prep(
        inputs["x"], inputs["edge_index"], inputs["edge_weight"], inputs["batch"]
    )
    key = (meta["C1"], meta["DP"])
    if key not in _CACHE:
        _CACHE[key] = build_program(*key)
    nc = _CACHE[key]

    W1 = np.asarray(inputs["W1"], dtype=np.float32)
    b1 = np.asarray(inputs["b1"], dtype=np.float32).reshape(128, 1)
    W2 = np.asarray(inputs["W2"], dtype=np.float32)
    wlc = np.asarray(inputs["Wlin"], dtype=np.float32).reshape(128, 1)
    wlr = np.asarray(inputs["Wlin"], dtype=np.float32).reshape(1, 128)
    blin = np.asarray(inputs["blin"], dtype=np.float32).reshape(1, 1)
    b2r = np.asarray(inputs["b2"], dtype=np.float32).reshape(1, 128)

    in_maps = []
    for c in range(NC):
        cr = dict(cores[c])
        cr.update(W1=W1, b1=b1, W2=W2, wlin_col=wlc, wlin_row=wlr, blin=blin, b2row=b2r)
        in_maps.append(cr)
    res = run_bass_kernel_spmd(nc, in_maps, list(range(NC)))
    out = np.asarray(res.results[0]["out"], dtype=np.float32).reshape(B, 1)
    return out
